# revision 26
# baseline (speedup 1.0000x reference)
"""Trainium2 Bass kernel for additive (Bahdanau-style) attention.

Reference computation (per batch b):
    inp  = x @ W_in.T + b_in                      # [H]
    ctx  = W_ctx @ context[b].T + b_ctx           # [H, S]
    att  = V . tanh(inp[:,None] + ctx)            # [S]
    att  = where(mask, -inf, att)
    alpha = softmax(att)
    hidden = ctx @ alpha                          # [H]
returns (hidden [B,H], att [B,S])

Sharding: data-parallel over batch B=32 across 8 cores (4 batches/core),
params replicated.

Per-core dataflow (bf16 fast path, f32 spine where cheap):
  1. SWDGE DMA loads context[b] [S,D] f32 from HBM, casting to bf16 in
     flight, into natural tiles ctx_nat [128(s%128), 64*128 (t,d)].
  2. HWDGE xbar transposes each [128s,128d] tile -> ctxT [128 d, S s].
  3. PE: ctx-matmuls, W_ctxT (bf16) stationary, rhs = ctxT chunks N=512.
  4. ACT: tanh(psum + (inp[b]+b_ctx)) via per-partition bias, out bf16.
  5. PE: att-matmuls, V stationary (M=1), rhs = tanh chunks -> att chunks.
  6. DVE: 0.5*att psum->sbuf; DMA redistributes [1,8192] -> [64,128].
  7. Masked -inf att output via (att/2 + mask*(-FLT_MAX))*2 overflow trick.
  8. Softmax with fixed shift C = sum|V| (no max pass needed):
     exp via ACT (scale=2, bias=-C, accum_out=rowsum), partition-sum via
     ones-matmul, reciprocal, scale -> alpha (bf16).
  9. hidden = W_ctx @ (context[b].T @ alpha) + b_ctx: 64 accumulating
     K=128,N=128 matmuls with alpha columns (1-col weight loads) as
     stationary, then two tiny f32 matmuls.
"""

import numpy as np

B, S, D, H = 32, 8192, 128, 128
NCORES = 8
BPC = B // NCORES  # batches per core
NT = S // 128      # 64 s-tiles per batch
FLT_MAX = 3.4028234663852886e38

_cached = {}


def _build():
    import concourse.bass as bass
    import concourse.mybir as mybir
    import concourse.tile as tile
    import concourse.bacc as bacc
    from concourse import masks
    from contextlib import ExitStack

    f32 = mybir.dt.float32
    bf16 = mybir.dt.bfloat16
    u8 = mybir.dt.uint8
    AF = mybir.ActivationFunctionType
    OP = mybir.AluOpType

    nc = bacc.Bacc("TRN2", target_bir_lowering=False, num_devices=NCORES, dynamic_dma_scratch_size=32768)

    ctx_d = nc.declare_dram_parameter("ctx4", [BPC, S, D], f32, isOutput=False)
    x_d = nc.declare_dram_parameter("x4", [BPC, D], f32, isOutput=False)
    mask_d = nc.declare_dram_parameter("mask4", [BPC, S], u8, isOutput=False)
    Win_d = nc.declare_dram_parameter("W_in", [H, D], f32, isOutput=False)
    bin_d = nc.declare_dram_parameter("b_in", [H], f32, isOutput=False)
    Wctx_d = nc.declare_dram_parameter("W_ctx", [H, D], f32, isOutput=False)
    bctx_d = nc.declare_dram_parameter("b_ctx", [H], f32, isOutput=False)
    V_d = nc.declare_dram_parameter("V", [H], f32, isOutput=False)
    hid_d = nc.declare_dram_parameter("hidden4", [BPC, H], f32, isOutput=True)
    att_d = nc.declare_dram_parameter("att4", [BPC, S], f32, isOutput=True)

    with tile.TileContext(nc) as tc, ExitStack() as ctx:
        const = ctx.enter_context(tc.tile_pool(name="const", bufs=1))
        ldpool = ctx.enter_context(tc.tile_pool(name="ld", bufs=4))
        ctxTpool = ctx.enter_context(tc.tile_pool(name="ctxT", bufs=3))
        tanhpool = ctx.enter_context(tc.tile_pool(name="tanh", bufs=3))
        attpool = ctx.enter_context(tc.tile_pool(name="attp", bufs=2))
        smpool = ctx.enter_context(tc.tile_pool(name="smp", bufs=2))
        psA = ctx.enter_context(tc.tile_pool(name="psA", bufs=2, space="PSUM"))
        psB = ctx.enter_context(tc.tile_pool(name="psB", bufs=2, space="PSUM"))
        psC = ctx.enter_context(tc.tile_pool(name="psC", bufs=2, space="PSUM"))

        # ---------------- init: constants and tiny precomputes ----------------
        ident = const.tile([128, 128], f32, tag="ident")
        masks.make_identity(nc, ident[:])

        ones128 = const.tile([128, 128], f32, tag="ones128")
        nc.gpsimd.memset(ones128[:], 1.0)
        one11 = const.tile([1, 1], f32, tag="one11")
        nc.gpsimd.memset(one11[:], 1.0)

        Win_sb = const.tile([H, D], f32, tag="Win")
        nc.sync.dma_start(Win_sb[:], Win_d[:, :])
        Wctx_sb = const.tile([H, D], f32, tag="Wctx")
        nc.sync.dma_start(Wctx_sb[:], Wctx_d[:, :])
        x_sb = const.tile([BPC, D], f32, tag="xsb")
        nc.sync.dma_start(x_sb[:], x_d[:, :])
        bin_row = const.tile([1, H], f32, tag="binrow")
        nc.sync.dma_start(bin_row[:], bin_d[:])
        bctx_row = const.tile([1, H], f32, tag="bctxrow")
        nc.sync.dma_start(bctx_row[:], bctx_d[:])
        v_row = const.tile([1, H], f32, tag="vrow")
        nc.sync.dma_start(v_row[:], V_d[:])

        # transpose W_in, W_ctx via PE (out = lhsT.T @ I)
        WinT_ps = psC.tile([128, 128], f32, tag="smallps")
        nc.tensor.matmul(WinT_ps[:], lhsT=Win_sb[:], rhs=ident[:], start=True, stop=True)
        WinT_sb = const.tile([D, H], f32, tag="WinT")
        nc.vector.tensor_copy(WinT_sb[:], WinT_ps[:])

        WctxT_ps = psC.tile([128, 128], f32, tag="smallps")
        nc.tensor.matmul(WctxT_ps[:], lhsT=Wctx_sb[:], rhs=ident[:], start=True, stop=True)
        WctxT_sb = const.tile([D, H], f32, tag="WctxT")
        nc.vector.tensor_copy(WctxT_sb[:], WctxT_ps[:])
        WctxT_bf = const.tile([D, H], bf16, tag="WctxTbf")
        nc.vector.tensor_copy(WctxT_bf[:], WctxT_ps[:])

        # column versions of b_in, b_ctx, V  ([1,128] -> [128,1] via PE)
        def to_col(row_ap, tag):
            ps = psC.tile([128, 1], f32, tag="smallps")
            nc.tensor.matmul(ps[:], lhsT=row_ap, rhs=one11[:], start=True, stop=True)
            col = const.tile([128, 1], f32, tag=tag)
            nc.vector.tensor_copy(col[:], ps[:])
            return col

        bin_col = to_col(bin_row[:], "bincol")
        bctx_col = to_col(bctx_row[:], "bctxcol")
        v_col = to_col(v_row[:], "vcol")

        v_bf = const.tile([128, 1], bf16, tag="vbf")
        nc.vector.tensor_copy(v_bf[:], v_col[:])

        # V replicated into columns 0/32/64/96 for 4-way col-tiled att matmuls
        v4 = const.tile([128, 128], bf16, tag="v4")
        nc.gpsimd.memset(v4[:], 0.0)
        for j in range(4):
            nc.vector.tensor_copy(v4[:, 32 * j : 32 * j + 1], v_col[:])

        # C = sum(|V|), broadcast to all partitions via ones-matmul
        absv = const.tile([128, 1], f32, tag="absv")
        nc.scalar.activation(absv[:], v_col[:], AF.Abs)
        C_ps = psC.tile([128, 1], f32, tag="smallps")
        nc.tensor.matmul(C_ps[:], lhsT=ones128[:], rhs=absv[:], start=True, stop=True)
        negC = const.tile([128, 1], f32, tag="negC")
        nc.vector.tensor_scalar_mul(negC[:], C_ps[:], -1.0)

        # xT [D, BPC] then inp = W_in @ x.T  -> [H, BPC]
        xT_ps = psC.tile([128, BPC], f32, tag="smallps")
        nc.tensor.matmul(xT_ps[:], lhsT=x_sb[:], rhs=ident[0:BPC, 0:BPC], start=True, stop=True)
        xT_sb = const.tile([D, BPC], f32, tag="xT")
        nc.vector.tensor_copy(xT_sb[:], xT_ps[:])
        inp_ps = psC.tile([128, BPC], f32, tag="smallps")
        nc.tensor.matmul(inp_ps[:], lhsT=WinT_sb[:], rhs=xT_sb[:], start=True, stop=True)
        # bias4[h, b] = inp[h, b] + b_in[h] + b_ctx[h]
        bias4 = const.tile([H, BPC], f32, tag="bias4")
        nc.vector.tensor_scalar(
            bias4[:], inp_ps[:], bin_col[:], bctx_col[:], op0=OP.add, op1=OP.add
        )

        ident_bf = const.tile([128, 128], bf16, tag="identbf")
        nc.vector.tensor_copy(ident_bf[:], ident[:])

        hidden_acc = const.tile([128, BPC], f32, tag="hidacc")

        # ---------------- per-batch pipeline ----------------
        # s-index layout (all natural order): ctx_nat [128, (t d)]: partition p,
        # free t*128+d = context[b, t*128+p, d]. ctxT free index i = s directly.
        #
        # Emission is phase-ordered so each in-order sequencer's stream never
        # blocks later independent work: all loads+transposes first (gpsimd /
        # sync), then per-batch compute, with tail DMAs queued behind the
        # transposes on sync.
        ctx_nats, ctxTs, masks_t, alphaPs = [], [], [], []
        for b in range(BPC):
            ctx_nat = ldpool.tile([128, S], bf16, tag="ctxnat")
            ctxT = ctxTpool.tile([128, S], bf16, tag="ctxT")
            nc.gpsimd.dma_start(
                ctx_nat.rearrange("p (t d) -> p t d", d=D),
                ctx_d[b].rearrange("(t p) d -> p t d", p=128),
            )
            mask_t = smpool.tile([64, 128], u8, tag="maskt")
            nc.scalar.dma_start(mask_t[:], mask_d[b].rearrange("(p f) -> p f", p=64))
            ctx_nats.append(ctx_nat)
            ctxTs.append(ctxT)
            masks_t.append(mask_t)

        def emit_transpose(k):
            nc.sync.dma_start(
                ctxTs[k].rearrange("d (t s) -> d t s", s=128),
                ctx_nats[k][:],
                transpose=True,
            )

        emit_transpose(0)
        emit_transpose(1)

        for b in range(BPC):
            ctx_nat, ctxT, mask_t = ctx_nats[b], ctxTs[b], masks_t[b]
            maskf = smpool.tile([64, 128], f32, tag="maskf")
            nc.vector.tensor_copy(maskf[:], mask_t[:])
            msc_out = smpool.tile([64, 128], f32, tag="mscout")
            nc.vector.tensor_scalar_mul(msc_out[:], maskf[:], -FLT_MAX)
            msc_sm = smpool.tile([64, 128], f32, tag="mscsm")
            nc.vector.tensor_scalar_mul(msc_sm[:], maskf[:], -5000.0)

            # ctx matmuls + tanh + 4-way col-tiled att matmuls
            att_sp = attpool.tile([128, 2048], f32, tag="attsp")
            for g in range(4):  # 4 passes x 2048 s
                tanhs = []
                for h2 in range(2):
                    ps_ctx = psA.tile([128, 1024], f32, tag="ctxps")
                    for q2 in range(2):
                        c = g * 4 + h2 * 2 + q2  # 512-chunk of s
                        nc.tensor.matmul(
                            ps_ctx[:, q2 * 512 : (q2 + 1) * 512],
                            lhsT=WctxT_bf[:],
                            rhs=ctxT[:, c * 512 : (c + 1) * 512],
                            start=True,
                            stop=True,
                        )
                    tanh_t = tanhpool.tile([128, 1024], bf16, tag="tanht")
                    nc.scalar.activation(
                        tanh_t[:],
                        ps_ctx[:],
                        AF.Tanh,
                        bias=bias4[:, b : b + 1],
                        scale=1.0,
                    )
                    tanhs.append(tanh_t)
                ps_att4 = psB.tile([128, 512], f32, tag="attps")
                for j in range(4):
                    nc.tensor.matmul(
                        ps_att4[32 * j : 32 * j + 1, :],
                        lhsT=v4[:, 32 * j : 32 * j + 1],
                        rhs=tanhs[j // 2][:, (j % 2) * 512 : (j % 2 + 1) * 512],
                        start=True,
                        stop=True,
                        tile_position=(0, 32 * j),
                    )
                # 0.5 * att, psum -> sbuf; rows other than 0/32/64/96 are
                # garbage and never read downstream
                nc.vector.tensor_scalar_mul(
                    att_sp[:, g * 512 : (g + 1) * 512],
                    ps_att4[:],
                    0.5,
                )

            # redistribute att/2 -> [64, 128] (s = 128p + f): 16 block copies,
            # chunk c=4g+j at att_sp[32j, 512g:...] -> partitions [4c, 4c+4)
            att64h = smpool.tile([64, 128], f32, tag="att64h")
            for g in range(4):
                for j in range(4):
                    c = 4 * g + j
                    eng = nc.sync if c % 2 == 0 else nc.scalar
                    eng.dma_start(
                        att64h[4 * c : 4 * c + 4, :],
                        att_sp[
                            32 * j : 32 * j + 1, 512 * g : 512 * (g + 1)
                        ].rearrange("o (z f) -> o z f", f=128),
                    )

            # masked att output: (att/2 + mask*(-FLT_MAX)) * 2 -> -inf at mask
            att_sum = smpool.tile([64, 128], f32, tag="attsum")
            nc.vector.tensor_add(att_sum[:], att64h[:], msc_out[:])
            att_out = smpool.tile([64, 128], f32, tag="attout")
            nc.vector.tensor_scalar_mul(att_out[:], att_sum[:], 2.0)
            nc.sync.dma_start(att_d[b].rearrange("(p f) -> p f", p=64), att_out[:])

            # softmax: exp(att - 1e4*mask - C), rowsum via accum_out
            sm_in = smpool.tile([64, 128], f32, tag="smin")
            nc.vector.tensor_add(sm_in[:], att64h[:], msc_sm[:])
            exp_t = smpool.tile([64, 128], f32, tag="expt")
            sumcol = smpool.tile([64, 1], f32, tag="sumcol")
            nc.scalar.activation(
                exp_t[:],
                sm_in[:],
                AF.Exp,
                bias=negC[0:64, :],
                scale=2.0,
                accum_out=sumcol[:],
            )
            tot_ps = psC.tile([128, 1], f32, tag="smallps")
            nc.tensor.matmul(
                tot_ps[:], lhsT=ones128[0:64, :], rhs=sumcol[:], start=True, stop=True
            )
            recip = smpool.tile([128, 1], f32, tag="recip")
            nc.vector.reciprocal(recip[:], tot_ps[:])
            alpha64_bf = smpool.tile([64, 128], bf16, tag="alphabf")
            nc.vector.tensor_scalar_mul(alpha64_bf[:], exp_t[:], recip[0:64, :])
            alphaT_ps = psB.tile([128, 64], bf16, tag="attps")
            nc.tensor.transpose(
                alphaT_ps[:], alpha64_bf[:], ident_bf[0:64, 0:64]
            )
            alphaP = smpool.tile([128, 64], bf16, tag="alphaP", bufs=4)
            nc.vector.tensor_copy(alphaP[:], alphaT_ps[:])
            alphaPs.append(alphaP)
            if b + 2 < BPC:
                emit_transpose(b + 2)

        # ---------------- final phase: csum + hidden (PE stream stays free
        # of softmax-gated matmuls during the main batch pipeline) ----------
        for b in range(BPC):
            ctx_nat, alphaP = ctx_nats[b], alphaPs[b]
            csum_ps = psC.tile([1, 128], f32, tag="smallps")
            for t in range(NT):
                nc.tensor.matmul(
                    csum_ps[:],
                    lhsT=alphaP[:, t : t + 1],
                    rhs=ctx_nat[:, t * 128 : (t + 1) * 128],
                    start=(t == 0),
                    stop=(t == NT - 1),
                )
            csum_sb = smpool.tile([1, 128], f32, tag="csumsb")
            nc.vector.tensor_copy(csum_sb[:], csum_ps[:])
            csumT_ps = psC.tile([128, 1], f32, tag="smallps")
            nc.tensor.matmul(
                csumT_ps[:], lhsT=csum_sb[:], rhs=one11[:], start=True, stop=True
            )
            csum_col = smpool.tile([128, 1], f32, tag="csumcol")
            nc.vector.tensor_copy(csum_col[:], csumT_ps[:])
            hid_ps = psC.tile([128, 1], f32, tag="smallps")
            nc.tensor.matmul(
                hid_ps[:], lhsT=WctxT_sb[:], rhs=csum_col[:], start=True, stop=True
            )
            nc.vector.tensor_scalar_add(
                hidden_acc[:, b : b + 1], hid_ps[:], bctx_col[:]
            )

        # ---------------- epilogue: hidden [128, BPC] -> [BPC, 128] ----------------
        hidT_ps = psC.tile([BPC, 128], f32, tag="smallps")
        nc.tensor.matmul(
            hidT_ps[:], lhsT=hidden_acc[:], rhs=ident[:], start=True, stop=True
        )
        hidT_sb = smpool.tile([BPC, 128], f32, tag="hidT")
        nc.vector.tensor_copy(hidT_sb[:], hidT_ps[:])
        nc.sync.dma_start(hid_d[:, :], hidT_sb[:])

    nc.compile()
    return nc


def _get_nc():
    if "nc" not in _cached:
        _cached["nc"] = _build()
    return _cached["nc"]


def kernel(x, context, mask, W_in, b_in, W_ctx, b_ctx, V, trace=False, tmpdir=None):
    from concourse.bass_utils import run_bass_kernel_spmd

    x = np.asarray(x, dtype=np.float32)
    context = np.asarray(context, dtype=np.float32)
    mask_u8 = np.asarray(mask).astype(np.uint8)
    W_in = np.asarray(W_in, dtype=np.float32)
    b_in = np.asarray(b_in, dtype=np.float32)
    W_ctx = np.asarray(W_ctx, dtype=np.float32)
    b_ctx = np.asarray(b_ctx, dtype=np.float32)
    V = np.asarray(V, dtype=np.float32)

    nc = _get_nc()
    in_maps = []
    for c in range(NCORES):
        sl = slice(BPC * c, BPC * (c + 1))
        in_maps.append(
            {
                "ctx4": np.ascontiguousarray(context[sl]),
                "x4": np.ascontiguousarray(x[sl]),
                "mask4": np.ascontiguousarray(mask_u8[sl]),
                "W_in": W_in,
                "b_in": b_in,
                "W_ctx": W_ctx,
                "b_ctx": b_ctx,
                "V": V,
            }
        )
    res = run_bass_kernel_spmd(
        nc, in_maps, list(range(NCORES)), trace=trace, tmpdir=tmpdir
    )
    _cached["last_results"] = res
    hidden = np.concatenate([res.results[c]["hidden4"] for c in range(NCORES)], axis=0)
    att = np.concatenate([res.results[c]["att4"] for c in range(NCORES)], axis=0)
    return hidden, att


if __name__ == "__main__":
    import reference

    inputs = {k: np.asarray(v) for k, v in reference.setup_inputs().items()}
    out = kernel(**inputs)
    print([o.shape for o in out])


# revision 27
# speedup vs baseline: 1.0181x; 1.0181x over previous
"""Trainium2 Bass kernel for additive (Bahdanau-style) attention.

Reference computation (per batch b):
    inp  = x @ W_in.T + b_in                      # [H]
    ctx  = W_ctx @ context[b].T + b_ctx           # [H, S]
    att  = V . tanh(inp[:,None] + ctx)            # [S]
    att  = where(mask, -inf, att)
    alpha = softmax(att)
    hidden = ctx @ alpha                          # [H]
returns (hidden [B,H], att [B,S])

Sharding: data-parallel over batch B=32 across 8 cores (4 batches/core),
params replicated.

Per-core dataflow (bf16 fast path, f32 spine where cheap):
  1. SWDGE DMA loads context[b] [S,D] f32 from HBM, casting to bf16 in
     flight, into natural tiles ctx_nat [128(s%128), 64*128 (t,d)].
  2. HWDGE xbar transposes each [128s,128d] tile -> ctxT [128 d, S s].
  3. PE: ctx-matmuls, W_ctxT (bf16) stationary, rhs = ctxT chunks N=512.
  4. ACT: tanh(psum + (inp[b]+b_ctx)) via per-partition bias, out bf16.
  5. PE: att-matmuls, V stationary (M=1), rhs = tanh chunks -> att chunks.
  6. DVE: 0.5*att psum->sbuf; DMA redistributes [1,8192] -> [64,128].
  7. Masked -inf att output via (att/2 + mask*(-FLT_MAX))*2 overflow trick.
  8. Softmax with fixed shift C = sum|V| (no max pass needed):
     exp via ACT (scale=2, bias=-C, accum_out=rowsum), partition-sum via
     ones-matmul, reciprocal, scale -> alpha (bf16).
  9. hidden = W_ctx @ (context[b].T @ alpha) + b_ctx: 64 accumulating
     K=128,N=128 matmuls with alpha columns (1-col weight loads) as
     stationary, then two tiny f32 matmuls.
"""

import numpy as np

B, S, D, H = 32, 8192, 128, 128
NCORES = 8
BPC = B // NCORES  # batches per core
NT = S // 128      # 64 s-tiles per batch
FLT_MAX = 3.4028234663852886e38

_cached = {}


def _build():
    import concourse.bass as bass
    import concourse.mybir as mybir
    import concourse.tile as tile
    import concourse.bacc as bacc
    from concourse import masks
    from contextlib import ExitStack

    f32 = mybir.dt.float32
    bf16 = mybir.dt.bfloat16
    u8 = mybir.dt.uint8
    AF = mybir.ActivationFunctionType
    OP = mybir.AluOpType

    nc = bacc.Bacc("TRN2", target_bir_lowering=False, num_devices=NCORES, dynamic_dma_scratch_size=32768)

    ctx_d = nc.declare_dram_parameter("ctx4", [BPC, S, D], f32, isOutput=False)
    x_d = nc.declare_dram_parameter("x4", [BPC, D], f32, isOutput=False)
    mask_d = nc.declare_dram_parameter("mask4", [BPC, S], u8, isOutput=False)
    Win_d = nc.declare_dram_parameter("W_in", [H, D], f32, isOutput=False)
    bin_d = nc.declare_dram_parameter("b_in", [H], f32, isOutput=False)
    Wctx_d = nc.declare_dram_parameter("W_ctx", [H, D], f32, isOutput=False)
    bctx_d = nc.declare_dram_parameter("b_ctx", [H], f32, isOutput=False)
    V_d = nc.declare_dram_parameter("V", [H], f32, isOutput=False)
    hid_d = nc.declare_dram_parameter("hidden4", [BPC, H], f32, isOutput=True)
    att_d = nc.declare_dram_parameter("att4", [BPC, S], f32, isOutput=True)

    with tile.TileContext(nc) as tc, ExitStack() as ctx:
        const = ctx.enter_context(tc.tile_pool(name="const", bufs=1))
        ldpool = ctx.enter_context(tc.tile_pool(name="ld", bufs=4))
        ctxTpool = ctx.enter_context(tc.tile_pool(name="ctxT", bufs=3))
        tanhpool = ctx.enter_context(tc.tile_pool(name="tanh", bufs=3))
        attpool = ctx.enter_context(tc.tile_pool(name="attp", bufs=2))
        smpool = ctx.enter_context(tc.tile_pool(name="smp", bufs=2))
        psA = ctx.enter_context(tc.tile_pool(name="psA", bufs=2, space="PSUM"))
        psB = ctx.enter_context(tc.tile_pool(name="psB", bufs=2, space="PSUM"))
        psC = ctx.enter_context(tc.tile_pool(name="psC", bufs=2, space="PSUM"))

        # ---------------- init: constants and tiny precomputes ----------------
        ident = const.tile([128, 128], f32, tag="ident")
        masks.make_identity(nc, ident[:])

        ones128 = const.tile([128, 128], f32, tag="ones128")
        nc.gpsimd.memset(ones128[:], 1.0)
        one11 = const.tile([1, 1], f32, tag="one11")
        nc.gpsimd.memset(one11[:], 1.0)

        Win_sb = const.tile([H, D], f32, tag="Win")
        nc.sync.dma_start(Win_sb[:], Win_d[:, :])
        Wctx_sb = const.tile([H, D], f32, tag="Wctx")
        nc.sync.dma_start(Wctx_sb[:], Wctx_d[:, :])
        x_sb = const.tile([BPC, D], f32, tag="xsb")
        nc.sync.dma_start(x_sb[:], x_d[:, :])
        bin_row = const.tile([1, H], f32, tag="binrow")
        nc.sync.dma_start(bin_row[:], bin_d[:])
        bctx_row = const.tile([1, H], f32, tag="bctxrow")
        nc.sync.dma_start(bctx_row[:], bctx_d[:])
        v_row = const.tile([1, H], f32, tag="vrow")
        nc.sync.dma_start(v_row[:], V_d[:])

        # transpose W_in, W_ctx via PE (out = lhsT.T @ I)
        WinT_ps = psC.tile([128, 128], f32, tag="smallps")
        nc.tensor.matmul(WinT_ps[:], lhsT=Win_sb[:], rhs=ident[:], start=True, stop=True)
        WinT_sb = const.tile([D, H], f32, tag="WinT")
        nc.vector.tensor_copy(WinT_sb[:], WinT_ps[:])

        WctxT_ps = psC.tile([128, 128], f32, tag="smallps")
        nc.tensor.matmul(WctxT_ps[:], lhsT=Wctx_sb[:], rhs=ident[:], start=True, stop=True)
        WctxT_sb = const.tile([D, H], f32, tag="WctxT")
        nc.vector.tensor_copy(WctxT_sb[:], WctxT_ps[:])
        WctxT_bf = const.tile([D, H], bf16, tag="WctxTbf")
        nc.vector.tensor_copy(WctxT_bf[:], WctxT_ps[:])

        # column versions of b_in, b_ctx, V  ([1,128] -> [128,1] via PE)
        def to_col(row_ap, tag):
            ps = psC.tile([128, 1], f32, tag="smallps")
            nc.tensor.matmul(ps[:], lhsT=row_ap, rhs=one11[:], start=True, stop=True)
            col = const.tile([128, 1], f32, tag=tag)
            nc.vector.tensor_copy(col[:], ps[:])
            return col

        bin_col = to_col(bin_row[:], "bincol")
        bctx_col = to_col(bctx_row[:], "bctxcol")
        v_col = to_col(v_row[:], "vcol")

        v_bf = const.tile([128, 1], bf16, tag="vbf")
        nc.vector.tensor_copy(v_bf[:], v_col[:])

        # V replicated into columns 0/32/64/96 for 4-way col-tiled att matmuls
        v4 = const.tile([128, 128], bf16, tag="v4")
        nc.gpsimd.memset(v4[:], 0.0)
        for j in range(4):
            nc.vector.tensor_copy(v4[:, 32 * j : 32 * j + 1], v_col[:])

        # C = sum(|V|), broadcast to all partitions via ones-matmul
        absv = const.tile([128, 1], f32, tag="absv")
        nc.scalar.activation(absv[:], v_col[:], AF.Abs)
        C_ps = psC.tile([128, 1], f32, tag="smallps")
        nc.tensor.matmul(C_ps[:], lhsT=ones128[:], rhs=absv[:], start=True, stop=True)
        negC = const.tile([128, 1], f32, tag="negC")
        nc.vector.tensor_scalar_mul(negC[:], C_ps[:], -1.0)

        # xT [D, BPC] then inp = W_in @ x.T  -> [H, BPC]
        xT_ps = psC.tile([128, BPC], f32, tag="smallps")
        nc.tensor.matmul(xT_ps[:], lhsT=x_sb[:], rhs=ident[0:BPC, 0:BPC], start=True, stop=True)
        xT_sb = const.tile([D, BPC], f32, tag="xT")
        nc.vector.tensor_copy(xT_sb[:], xT_ps[:])
        inp_ps = psC.tile([128, BPC], f32, tag="smallps")
        nc.tensor.matmul(inp_ps[:], lhsT=WinT_sb[:], rhs=xT_sb[:], start=True, stop=True)
        # bias4[h, b] = inp[h, b] + b_in[h] + b_ctx[h]
        bias4 = const.tile([H, BPC], f32, tag="bias4")
        nc.vector.tensor_scalar(
            bias4[:], inp_ps[:], bin_col[:], bctx_col[:], op0=OP.add, op1=OP.add
        )

        ident_bf = const.tile([128, 128], bf16, tag="identbf")
        nc.vector.tensor_copy(ident_bf[:], ident[:])

        hidden_acc = const.tile([128, BPC], f32, tag="hidacc")

        # ---------------- per-batch pipeline ----------------
        # s-index layout (all natural order): ctx_nat [128, (t d)]: partition p,
        # free t*128+d = context[b, t*128+p, d]. ctxT free index i = s directly.
        #
        # Emission is phase-ordered so each in-order sequencer's stream never
        # blocks later independent work: all loads+transposes first (gpsimd /
        # sync), then per-batch compute, with tail DMAs queued behind the
        # transposes on sync.
        ctx_nats, ctxTs, masks_t, alphaPs = [], [], [], []
        for b in range(BPC):
            ctx_nat = ldpool.tile([128, S], bf16, tag="ctxnat")
            ctxT = ctxTpool.tile([128, S], bf16, tag="ctxT")
            nc.gpsimd.dma_start(
                ctx_nat.rearrange("p (t d) -> p t d", d=D),
                ctx_d[b].rearrange("(t p) d -> p t d", p=128),
            )
            mask_t = smpool.tile([64, 128], u8, tag="maskt")
            nc.scalar.dma_start(mask_t[:], mask_d[b].rearrange("(p f) -> p f", p=64))
            ctx_nats.append(ctx_nat)
            ctxTs.append(ctxT)
            masks_t.append(mask_t)

        def emit_transpose(k):
            nc.sync.dma_start(
                ctxTs[k].rearrange("d (t s) -> d t s", s=128),
                ctx_nats[k][:],
                transpose=True,
            )

        emit_transpose(0)
        emit_transpose(1)

        for b in range(BPC):
            ctx_nat, ctxT, mask_t = ctx_nats[b], ctxTs[b], masks_t[b]
            maskf = smpool.tile([64, 128], f32, tag="maskf")
            nc.vector.tensor_copy(maskf[:], mask_t[:])
            msc_out = smpool.tile([64, 128], f32, tag="mscout")
            nc.vector.tensor_scalar_mul(msc_out[:], maskf[:], -FLT_MAX)
            msc_sm = smpool.tile([64, 128], f32, tag="mscsm")
            nc.vector.tensor_scalar_mul(msc_sm[:], maskf[:], -5000.0)

            # ctx matmuls + tanh + 4-way col-tiled att matmuls
            att_sp = attpool.tile([128, 2048], f32, tag="attsp")
            for g in range(4):  # 4 passes x 2048 s
                tanhs = []
                for h2 in range(2):
                    ps_ctx = psA.tile([128, 1024], f32, tag="ctxps")
                    for q2 in range(2):
                        c = g * 4 + h2 * 2 + q2  # 512-chunk of s
                        nc.tensor.matmul(
                            ps_ctx[:, q2 * 512 : (q2 + 1) * 512],
                            lhsT=WctxT_bf[:],
                            rhs=ctxT[:, c * 512 : (c + 1) * 512],
                            start=True,
                            stop=True,
                        )
                    tanh_t = tanhpool.tile([128, 1024], bf16, tag="tanht")
                    nc.scalar.activation(
                        tanh_t[:],
                        ps_ctx[:],
                        AF.Tanh,
                        bias=bias4[:, b : b + 1],
                        scale=1.0,
                    )
                    tanhs.append(tanh_t)
                ps_att4 = psB.tile([128, 512], f32, tag="attps")
                for j in range(4):
                    nc.tensor.matmul(
                        ps_att4[32 * j : 32 * j + 1, :],
                        lhsT=v4[:, 32 * j : 32 * j + 1],
                        rhs=tanhs[j // 2][:, (j % 2) * 512 : (j % 2 + 1) * 512],
                        start=True,
                        stop=True,
                        tile_position=(0, 32 * j),
                    )
                # 0.5 * att, psum -> sbuf; rows other than 0/32/64/96 are
                # garbage and never read downstream
                nc.vector.tensor_scalar_mul(
                    att_sp[:, g * 512 : (g + 1) * 512],
                    ps_att4[:],
                    0.5,
                )

            # redistribute att/2 -> [64, 128] (s = 128p + f): 16 block copies,
            # chunk c=4g+j at att_sp[32j, 512g:...] -> partitions [4c, 4c+4)
            att64h = smpool.tile([64, 128], f32, tag="att64h")
            for g in range(4):
                for j in range(4):
                    c = 4 * g + j
                    nc.sync.dma_start(
                        att64h[4 * c : 4 * c + 4, :],
                        att_sp[
                            32 * j : 32 * j + 1, 512 * g : 512 * (g + 1)
                        ].rearrange("o (z f) -> o z f", f=128),
                    )

            # masked att output: (att/2 + mask*(-FLT_MAX)) * 2 -> -inf at mask
            att_sum = smpool.tile([64, 128], f32, tag="attsum")
            nc.vector.tensor_add(att_sum[:], att64h[:], msc_out[:])
            att_out = smpool.tile([64, 128], f32, tag="attout")
            nc.vector.tensor_scalar_mul(att_out[:], att_sum[:], 2.0)
            nc.sync.dma_start(att_d[b].rearrange("(p f) -> p f", p=64), att_out[:])

            # softmax: exp(att - 1e4*mask - C), rowsum via accum_out
            sm_in = smpool.tile([64, 128], f32, tag="smin")
            nc.vector.tensor_add(sm_in[:], att64h[:], msc_sm[:])
            exp_t = smpool.tile([64, 128], f32, tag="expt")
            sumcol = smpool.tile([64, 1], f32, tag="sumcol")
            nc.scalar.activation(
                exp_t[:],
                sm_in[:],
                AF.Exp,
                bias=negC[0:64, :],
                scale=2.0,
                accum_out=sumcol[:],
            )
            tot_ps = psC.tile([128, 1], f32, tag="smallps")
            nc.tensor.matmul(
                tot_ps[:], lhsT=ones128[0:64, :], rhs=sumcol[:], start=True, stop=True
            )
            recip = smpool.tile([128, 1], f32, tag="recip")
            nc.vector.reciprocal(recip[:], tot_ps[:])
            alpha64_bf = smpool.tile([64, 128], bf16, tag="alphabf")
            nc.vector.tensor_scalar_mul(alpha64_bf[:], exp_t[:], recip[0:64, :])
            alphaT_ps = psB.tile([128, 64], bf16, tag="attps")
            nc.tensor.transpose(
                alphaT_ps[:], alpha64_bf[:], ident_bf[0:64, 0:64]
            )
            alphaP = smpool.tile([128, 64], bf16, tag="alphaP", bufs=4)
            nc.vector.tensor_copy(alphaP[:], alphaT_ps[:])
            alphaPs.append(alphaP)
            if b + 2 < BPC:
                emit_transpose(b + 2)

        # ---------------- final phase: csum + hidden. All 4 batches' csums
        # land in one PSUM tile at rows {0,32,64,96} via col-tiling, then one
        # selector-matmul transposes them together.
        csum4_ps = psC.tile([128, 128], f32, tag="smallps")
        for b in range(BPC):
            ctx_nat, alphaP = ctx_nats[b], alphaPs[b]
            for t in range(NT):
                nc.tensor.matmul(
                    csum4_ps[32 * b : 32 * b + 1, :],
                    lhsT=alphaP[:, t : t + 1],
                    rhs=ctx_nat[:, t * 128 : (t + 1) * 128],
                    start=(t == 0),
                    stop=(t == NT - 1),
                    tile_position=(0, 32 * b),
                )
        csum4_sb = smpool.tile([128, 128], f32, tag="csum4sb")
        nc.vector.tensor_copy(csum4_sb[:], csum4_ps[:])
        # csumT[d, b] = csum4_sb[32b, d]; selector col b = identity col 32b
        csumT_ps = psC.tile([128, BPC], f32, tag="smallps")
        nc.tensor.matmul(
            csumT_ps[:],
            lhsT=csum4_sb[:],
            rhs=ident[:, 0:128:32],
            start=True,
            stop=True,
        )
        csumT_sb = smpool.tile([128, BPC], f32, tag="csumTsb")
        nc.vector.tensor_copy(csumT_sb[:], csumT_ps[:])
        hid_ps = psC.tile([128, BPC], f32, tag="smallps")
        nc.tensor.matmul(
            hid_ps[:], lhsT=WctxT_sb[:], rhs=csumT_sb[:], start=True, stop=True
        )
        nc.vector.tensor_scalar_add(hidden_acc[:], hid_ps[:], bctx_col[:])

        # ---------------- epilogue: hidden [128, BPC] -> [BPC, 128] ----------------
        hidT_ps = psC.tile([BPC, 128], f32, tag="smallps")
        nc.tensor.matmul(
            hidT_ps[:], lhsT=hidden_acc[:], rhs=ident[:], start=True, stop=True
        )
        hidT_sb = smpool.tile([BPC, 128], f32, tag="hidT")
        nc.vector.tensor_copy(hidT_sb[:], hidT_ps[:])
        nc.sync.dma_start(hid_d[:, :], hidT_sb[:])

    nc.compile()
    return nc


def _get_nc():
    if "nc" not in _cached:
        _cached["nc"] = _build()
    return _cached["nc"]


def kernel(x, context, mask, W_in, b_in, W_ctx, b_ctx, V, trace=False, tmpdir=None):
    from concourse.bass_utils import run_bass_kernel_spmd

    x = np.asarray(x, dtype=np.float32)
    context = np.asarray(context, dtype=np.float32)
    mask_u8 = np.asarray(mask).astype(np.uint8)
    W_in = np.asarray(W_in, dtype=np.float32)
    b_in = np.asarray(b_in, dtype=np.float32)
    W_ctx = np.asarray(W_ctx, dtype=np.float32)
    b_ctx = np.asarray(b_ctx, dtype=np.float32)
    V = np.asarray(V, dtype=np.float32)

    nc = _get_nc()
    in_maps = []
    for c in range(NCORES):
        sl = slice(BPC * c, BPC * (c + 1))
        in_maps.append(
            {
                "ctx4": np.ascontiguousarray(context[sl]),
                "x4": np.ascontiguousarray(x[sl]),
                "mask4": np.ascontiguousarray(mask_u8[sl]),
                "W_in": W_in,
                "b_in": b_in,
                "W_ctx": W_ctx,
                "b_ctx": b_ctx,
                "V": V,
            }
        )
    res = run_bass_kernel_spmd(
        nc, in_maps, list(range(NCORES)), trace=trace, tmpdir=tmpdir
    )
    _cached["last_results"] = res
    hidden = np.concatenate([res.results[c]["hidden4"] for c in range(NCORES)], axis=0)
    att = np.concatenate([res.results[c]["att4"] for c in range(NCORES)], axis=0)
    return hidden, att


if __name__ == "__main__":
    import reference

    inputs = {k: np.asarray(v) for k, v in reference.setup_inputs().items()}
    out = kernel(**inputs)
    print([o.shape for o in out])
